# revision 1
# baseline (speedup 1.0000x reference)
"""Trainium2 Bass kernel for nn_BinaryBlock (binary conv1d block).

Computation (numerically, after collapsing the STE identities):
    x_bin = where(x >= alpha, 1, -1)
    w_eff = weight * mean(|weight|, axis=(1,2), keepdims)
    y     = conv1d(x_bin, w_eff, stride 1, pad 1) + bias
    out   = RPReLU(y)  (= where(y > gamma, y - gamma + zeta, beta*(y-gamma) + zeta))

Strategy: data-parallel over batch B=32 across 8 cores (4 batches/core).
On-device, the +-1 input is recast as a {0,1} mask m = (x >= alpha):
    conv(2m-1, w) = conv(m, 2w) - S_all[co]  (+ S_k0[co] at l=0, S_k2[co] at l=L-1)
so the sign op is ONE tensor_scalar (is_ge) per x tile, and the correction
folds into the per-channel bias except for two boundary columns.

Mixed-precision PE schedule: masks are stored once as fp8e4 ({0,1} exact).
Conv taps k=0,1 are quantized to fp8e4 and run as DoubleRow matmuls (the
two ci-tiles paired in one 256-deep fp8 matmul at ~2 rows/cycle); tap k=2
stays fp16 (fp8 moving x fp16 stationary runs at bf16 speed), so each
[128,512] PSUM tile takes 4 matmuls (2 DR + 2 fp16) instead of 6 fp16.
Measured on the real inputs, the e4m3 tap quantization costs 1.6e-2
relative error (vs the 2e-2 gate); the k2 tap in fp16 keeps the margin.
The per-channel conv corrections (S_all/S_k0/S_k2) are computed host-side
from the QUANTIZED weights in f64, so the mask identity stays exact.
Weights are pre-scaled by PSCALE=2048 (power of 2) to center the e4m3
range and dodge fp16 denormals; the epilogue un-scales via the
activation's free `scale` operand.

Schedule: DMA issue costs ~0.65us per dma_start on a queue engine and
the DMA path crawls (~30GB/s) for its first few microseconds, so the
batch-0 x loads are issued first and chunked (first chunks small) on the
GpSimd queue; weights+constants are packed DMAs on the Scalar queue;
outputs store fp16, two l-tiles per DMA, on the Sync queue. A few
discarded matmuls on a zero tile (no weight dependency) warm the PE HAM
clock during the fill so the real stream starts at full rate. Epilogues
alternate Scalar/Vector engines.
"""

import numpy as np
import ml_dtypes

# Problem shape (hardcoded per contract)
B, C, L = 32, 256, 4096
K = 3
N_CORES = 8
B_PER_CORE = B // N_CORES          # 4
P = 128                            # partitions
CI_T = C // P                      # 2 input-channel tiles
CO_T = C // P                      # 2 output-channel tiles
NT = 512                           # matmul free dim / PSUM bank (fp32)
LT = L // NT                       # 8 l-tiles
LP = L + 2                         # padded mask length
LP8 = 4112                         # mask row stride (16-aligned for DR APs)
PSCALE = 2048.0                    # weight pre-scale (power of 2)
GRP = 4                            # l-tiles per psum group
# batch-0 x chunk boundaries, aligned so the first (b0,co0) psum groups
# (1,1,2,2,2 l-tiles) consume chunks in arrival order: a group ending at
# l-tile T needs x cols through T*512
XSPLITS = (513, 1025, 2049, 3073, 4096)
# Discarded HAM-warmup matmuls: ~427ns each (cold) bridge the DMA-wake
# window so the PE clock is warm when the real stream starts.
WARMUP = 7

_CACHE = {}


def _build(trivial, x_bf16_ok):
    """Build + compile the SPMD Bass program. Returns the Bacc module."""
    import concourse.bacc as bacc
    import concourse.mybir as mybir
    from concourse import tile

    f32 = mybir.dt.float32
    f16 = mybir.dt.float16
    bf16 = mybir.dt.bfloat16
    f8 = mybir.dt.float8e4
    x_dt = f8 if x_bf16_ok else f32
    Alu = mybir.AluOpType
    Act = mybir.ActivationFunctionType
    DR = mybir.MatmulPerfMode.DoubleRow

    nc = bacc.Bacc("TRN2", target_bir_lowering=False, debug=False,
                   num_devices=N_CORES)

    xb_d = nc.dram_tensor("xb", [B_PER_CORE * CI_T, P, L], x_dt,
                          kind="ExternalInput")
    # fp8 pair-weights for taps 0,1: [P(ci within tile), k, ci_t, co]
    w8_d = nc.dram_tensor("w8", [P, 2, CI_T, C], f8, kind="ExternalInput")
    # fp16 weights for tap 2: [P, ci_t, co]
    w16_d = nc.dram_tensor("w16", [P, CI_T, C], f16, kind="ExternalInput")
    # cvav columns: per co_t 8 cols (0=c1, 1=sk0, 2=sk2, 3=beta-1, 4=zeta),
    # then 2 cols of alpha (per ci_t)
    cvav_d = nc.dram_tensor("cvav", [P, 2 * 8 + CI_T], f32,
                            kind="ExternalInput")
    y_d = nc.dram_tensor("y", [B_PER_CORE, CO_T, P, L], f16,
                         kind="ExternalOutput")

    with tile.TileContext(nc) as tc:
        with (
            tc.tile_pool(name="wpool", bufs=1) as wpool,
            tc.tile_pool(name="cpool", bufs=1) as cpool,
            tc.tile_pool(name="xpool", bufs=4) as xpool,
            tc.tile_pool(name="mpool", bufs=3) as mpool,
            tc.tile_pool(name="opool", bufs=8) as opool,
            tc.tile_pool(name="upool", bufs=4) as upool,
            tc.tile_pool(name="psum", bufs=8, space="PSUM") as psum,
        ):
            # ---- batch-0 x loads first, chunked, on GpSimd ----
            # During the DMA wake window (~first 4us of transfers)
            # bandwidth is scarce and the HW round-robins packets across
            # active queues, so keep the early critical transfers on just
            # two queues: x chunks serialized on GpSimd, weights+consts on
            # Scalar.
            xt0 = [xpool.tile([P, L], x_dt, tag="x", name=f"x0_{ci}")
                   for ci in range(CI_T)]
            bounds = [0, *XSPLITS]
            for c in range(len(XSPLITS)):
                for ci in range(CI_T):
                    lo, hi = bounds[c], bounds[c + 1]
                    nc.gpsimd.dma_start(out=xt0[ci][:, lo:hi],
                                        in_=xb_d[ci, :, lo:hi])
            w8t = wpool.tile([P, 2, CI_T, C], f8, tag="w8", name="w8")
            nc.scalar.dma_start(out=w8t[:], in_=w8_d[:])
            w16t = wpool.tile([P, CI_T, C], f16, tag="w16", name="w16")
            nc.scalar.dma_start(out=w16t[:], in_=w16_d[:])
            ct = cpool.tile([P, 2 * 8 + CI_T], f32, tag="cv", name="cv")
            nc.scalar.dma_start(out=ct[:], in_=cvav_d[:])
            cv_sb = [ct[:, 8 * co:8 * co + 8] for co in range(CO_T)]
            av_sb = [ct[:, 16 + ci:17 + ci] for ci in range(CI_T)]

            # zero tile for PE warmup: FIRST op on Vector so the HAM
            # warmup matmuls start as early as possible
            if WARMUP:
                zt = mpool.tile([P, NT], f16, tag="z", name="z")
                nc.vector.memset(zt[:], 0.0)
            # ---- batch-0 masks, chunked (Vector), fp8 {0,1} ----
            mt0 = mpool.tile([P, CI_T, LP8], f8, tag="m", name="m0")
            for ci in range(CI_T):
                nc.vector.memset(mt0[:, ci, 0:1], 0.0)
                nc.vector.memset(mt0[:, ci, L + 1:L + 2], 0.0)
            for c in range(len(XSPLITS)):
                for ci in range(CI_T):
                    lo, hi = bounds[c], bounds[c + 1]
                    nc.vector.tensor_scalar(
                        mt0[:, ci, 1 + lo:1 + hi], xt0[ci][:, lo:hi],
                        av_sb[ci], None, Alu.is_ge)

            # ---- PE warmup: discarded matmuls on the zero tile ----
            if WARMUP:
                wu = psum.tile([P, NT], f32, tag="ps", name="wu")
                for _ in range(WARMUP):
                    nc.tensor.matmul(wu[:], zt[:, 0:P], zt[:],
                                     start=True, stop=True)

            # masks for batches 1..3 are produced in 2048-col chunks,
            # interleaved between psum-group epilogues so a long mask op
            # never blocks the engine queue ahead of a psum drain.
            # ci0 chunks run on Vector, ci1 on GpSimd (idle mid-batch).
            mt = mt0
            nxt = None          # (mask tile, [mask-op closures]) for b+1
            for b in range(B_PER_CORE):
                if b > 0:
                    mt, pend = nxt
                    for fn in pend:   # flush leftovers
                        fn()
                nxt = None
                pend = []
                if b + 1 < B_PER_CORE:
                    bn = b + 1
                    mn = mpool.tile([P, CI_T, LP8], f8, tag="m", name="m")
                    for ci in range(CI_T):
                        xt = xpool.tile([P, L], x_dt, tag="x", name="x")
                        nc.gpsimd.dma_start(out=xt[:],
                                            in_=xb_d[bn * CI_T + ci])
                        nc.vector.memset(mn[:, ci, 0:1], 0.0)
                        nc.vector.memset(mn[:, ci, L + 1:L + 2], 0.0)
                        # one full-row mask op per ci, slotted between
                        # co==1 group epilogues (x landed long before, so
                        # it never stalls the in-order Vector queue)
                        def chunk(mn=mn, ci=ci, xt=xt):
                            nc.vector.tensor_scalar(
                                mn[:, ci, 1:1 + L], xt[:],
                                av_sb[ci], None, Alu.is_ge)
                        pend.append(chunk)
                    nxt = (mn, pend)

                # weight sets per co: 2 DoubleRow (taps 0,1, ci
                # paired) + 2 fp16 (tap 2 per ci)
                wsets_by_co = [
                    ([("dr", k, w8t[:, k, :, co * P:(co + 1) * P])
                      for k in range(2)]
                     + [("f16", ci, w16t[:, ci, co * P:(co + 1) * P])
                        for ci in range(CI_T)])
                    for co in range(CO_T)
                ]
                # (co, first l-tile, tiles) schedule: batch 0 interleaves
                # co0/co1 over the same l-range so the PE has 2x work per
                # arriving x chunk during the DMA wake; steady batches run
                # 4-tile groups; the very end tapers for a short drain
                if b == 0:
                    sched = [(0, 0, 1), (1, 0, 1), (0, 1, 1), (1, 1, 1),
                             (0, 2, 2), (1, 2, 2), (0, 4, 2), (1, 4, 2),
                             (0, 6, 2), (1, 6, 2)]
                elif b == B_PER_CORE - 1:
                    sched = [(0, 0, 4), (0, 4, 4), (1, 0, 4),
                             (1, 4, 2), (1, 6, 1), (1, 7, 1)]
                else:
                    sched = [(0, 0, 4), (0, 4, 4), (1, 0, 4), (1, 4, 4)]
                for (co, g0, grp) in sched:
                    cv = cv_sb[co]
                    wsets = wsets_by_co[co]
                    lt0 = g0 + grp
                    if True:
                        pts = [psum.tile([P, NT], f32, tag="ps", name="ps")
                               for _ in range(grp)]
                        # tile-major: each psum tile finishes its 4
                        # accumulating matmuls consecutively, so its
                        # epilogue starts ~3 tiles earlier than with
                        # weight-major order (LDWEIGHTS is re-issued per
                        # matmul either way, so tile-major costs nothing)
                        for j in range(grp):
                            for wi, (kind, koff, lhsT) in enumerate(wsets):
                                s = (g0 + j) * NT
                                st = (wi == 0)
                                sp = (wi == len(wsets) - 1)
                                if kind == "dr":
                                    nc.tensor.matmul(
                                        pts[j][:], lhsT,
                                        mt[:, :, s + koff:s + koff + NT],
                                        start=st, stop=sp, perf_mode=DR)
                                else:
                                    nc.tensor.matmul(
                                        pts[j][:], lhsT,
                                        mt[:, koff, s + 2:s + 2 + NT],
                                        start=st, stop=sp)
                        # epilogue: alternate Scalar/Vector; 2-tile stores
                        last_grp = (b == B_PER_CORE - 1 and co == CO_T - 1
                                    and lt0 == LT)
                        stg = 1 if last_grp else min(2, grp)
                        for half in range(grp // stg):
                            ot = opool.tile([P, stg * NT], f16, tag="o",
                                            name="o")
                            for jj in range(stg):
                                j = half * stg + jj
                                l_t = g0 + j
                                dst = ot[:, jj * NT:(jj + 1) * NT]
                                if trivial:
                                    # Scalar takes 3 of 4 epilogues (Vector
                                    # also carries the mask ops); the final
                                    # tapered groups drain on the faster DVE
                                    # so the post-last-matmul chain is short
                                    if j % 4 != 3 and not last_grp:
                                        nc.scalar.activation(
                                            dst, pts[j][:], Act.Identity,
                                            bias=cv[:, 0:1],
                                            scale=1.0 / PSCALE)
                                    else:
                                        nc.vector.tensor_scalar(
                                            dst, pts[j][:], 1.0 / PSCALE,
                                            cv[:, 0:1], Alu.mult, Alu.add)
                                    if l_t == 0:
                                        nc.vector.tensor_scalar(
                                            ot[:, 0:1], ot[:, 0:1],
                                            cv[:, 1:2], None, Alu.add)
                                    if l_t == LT - 1:
                                        e = stg * NT
                                        nc.vector.tensor_scalar(
                                            ot[:, e - 1:e], ot[:, e - 1:e],
                                            cv[:, 2:3], None, Alu.add)
                                else:
                                    # u = psum/PSCALE + c1 (+ boundary);
                                    # out = u + zeta + (beta-1)*min(u, 0)
                                    ut = upool.tile([P, NT], f32, tag="u",
                                                    name="u")
                                    nc.scalar.activation(
                                        ut[:], pts[j][:], Act.Identity,
                                        bias=cv[:, 0:1], scale=1.0 / PSCALE)
                                    if l_t == 0:
                                        nc.vector.tensor_scalar(
                                            ut[:, 0:1], ut[:, 0:1],
                                            cv[:, 1:2], None, Alu.add)
                                    if l_t == LT - 1:
                                        nc.vector.tensor_scalar(
                                            ut[:, NT - 1:NT],
                                            ut[:, NT - 1:NT],
                                            cv[:, 2:3], None, Alu.add)
                                    nt_ = upool.tile([P, NT], f32, tag="n",
                                                     name="n")
                                    nc.vector.tensor_scalar(
                                        nt_[:], ut[:], 0.0, cv[:, 3:4],
                                        Alu.min, Alu.mult)
                                    nc.vector.tensor_scalar(
                                        ut[:], ut[:], cv[:, 4:5], None,
                                        Alu.add)
                                    nc.vector.tensor_tensor(
                                        dst, ut[:], nt_[:], Alu.add)
                            lo = (g0 + half * stg) * NT
                            nc.sync.dma_start(
                                out=y_d[b, co, :, lo:lo + stg * NT],
                                in_=ot[:])
                        # slot one next-batch mask op between groups
                        # (for b0 only late ones, after b+1's x landed)
                        if co == CO_T - 1 and (b > 0 or g0 >= 4) and pend:
                            pend.pop(0)()

    nc.compile()
    return nc


def _host_prep(inputs):
    x = np.asarray(inputs["x"], dtype=np.float32)
    alpha = np.asarray(inputs["alpha"], dtype=np.float32).reshape(C)
    weight = np.asarray(inputs["weight"], dtype=np.float32)
    bias = np.asarray(inputs["bias"], dtype=np.float32).reshape(C)
    beta = np.asarray(inputs["beta"], dtype=np.float32).reshape(C)
    gamma = np.asarray(inputs["gamma"], dtype=np.float32).reshape(C)
    zeta = np.asarray(inputs["zeta"], dtype=np.float32).reshape(C)

    # Host-side weight prep (f32, matching the reference's f32 arithmetic)
    scale = np.mean(np.abs(weight), axis=(1, 2), dtype=np.float32)
    w_eff = weight * scale[:, None, None]              # [co, ci, k] f32
    w2 = w_eff * (2.0 * PSCALE)                        # conv(m, 2w) form

    # quantize: taps 0,1 -> e4m3 (DoubleRow), tap 2 -> fp16
    w8 = w2[:, :, 0:2].astype(ml_dtypes.float8_e4m3)   # [co, ci, k01]
    w16 = w2[:, :, 2].astype(np.float16)               # [co, ci]
    # exact dequantized values for the conv corrections
    wq = np.empty_like(w_eff, dtype=np.float64)
    wq[:, :, 0:2] = w8.astype(np.float64) / (2.0 * PSCALE)
    wq[:, :, 2] = w16.astype(np.float64) / (2.0 * PSCALE)

    # pack fp8 pair-weights: [P(ci within tile), k, ci_t, co]
    w8p = np.ascontiguousarray(
        w8.transpose(1, 2, 0)                          # [ci, k, co]
        .reshape(CI_T, P, 2, C)                        # [ci_t, P, k, co]
        .transpose(1, 2, 0, 3))                        # [P, k, ci_t, co]
    # pack fp16 tap-2 weights: [P, ci_t, co]
    w16p = np.ascontiguousarray(
        w16.transpose(1, 0).reshape(CI_T, P, C).transpose(1, 0, 2))

    S_all = wq.sum(axis=(1, 2))                        # [co]
    S_k0 = wq[:, :, 0].sum(axis=1)
    S_k2 = wq[:, :, 2].sum(axis=1)

    trivial = bool(np.all(beta == 1.0))
    c1 = (bias - gamma - S_all).astype(np.float32)
    if trivial:
        c1 = (c1 + zeta).astype(np.float32)
    cv = np.zeros((CO_T, P, 8), dtype=np.float32)
    cv[:, :, 0] = c1.reshape(CO_T, P)
    cv[:, :, 1] = S_k0.astype(np.float32).reshape(CO_T, P)
    cv[:, :, 2] = S_k2.astype(np.float32).reshape(CO_T, P)
    cv[:, :, 3] = (beta - 1.0).reshape(CO_T, P)
    cv[:, :, 4] = zeta.reshape(CO_T, P)
    cvav = np.zeros((P, 2 * 8 + CI_T), dtype=np.float32)
    cvav[:, 0:8] = cv[0]
    cvav[:, 8:16] = cv[1]
    cvav[:, 16:16 + CI_T] = alpha.reshape(CI_T, P).T

    x_bf16_ok = bool(np.all(alpha == 0.0))
    if x_bf16_ok:
        xs = x.reshape(N_CORES, B_PER_CORE * CI_T, P, L)
        xs = xs.astype(ml_dtypes.float8_e4m3)
        wrong = (xs.astype(np.float32) == 0.0) & (
            x.reshape(xs.shape) < 0.0)
        xs[wrong] = ml_dtypes.float8_e4m3(-0.001953125)
    else:
        xs = x.reshape(N_CORES, B_PER_CORE * CI_T, P, L)

    in_maps = [{"xb": xs[i], "w8": w8p, "w16": w16p, "cvav": cvav}
               for i in range(N_CORES)]
    return in_maps, (trivial, x_bf16_ok)


def kernel(**inputs):
    from concourse.bass_utils import run_bass_kernel_spmd

    in_maps, key = _host_prep(inputs)
    if key not in _CACHE:
        _CACHE[key] = _build(*key)
    nc = _CACHE[key]

    res = run_bass_kernel_spmd(nc, in_maps, list(range(N_CORES)))
    out = np.concatenate(
        [r["y"].reshape(B_PER_CORE, C, L) for r in res.results], axis=0)
    return out.astype(np.float32)

